# revision 31
# baseline (speedup 1.0000x reference)
"""Trainium2 Bass kernel for CSAM channel self-attention module.

Per batch b (one per NeuronCore, B=8 over 8 cores):
    v      = x2[b].reshape(7, D)                 # D = 64*128*128 = 1048576
    E      = v @ v.T                             # [7,7] gram ("energy")
    att    = softmax(rowmax(E) - E, axis=-1)     # == exp(rowmin(E)-E)/Z
    out    = att @ v
    y[b]   = x1[b] * (gamma*out) + x1[b] = x1[b] * (gamma*out + 1)

Layout: d = q*65536 + w*2048 + f  (Q=16 runs, stream tiles [112, 2048] with
partition p = 7*q + m and 8KB contiguous DRAM lines).

Pass A: stream x2 as FULL [112, 8KB-line] DMAs alternating the two HWDGE
queues (SP + ACT; 8KB lines sustain ~390+ GB/s where 4KB lines cap at
~250), cast fp32 -> fp16 cache tiles [113, 2048] in one GPSIMD op per tile
(keeps the DVE stream free and decouples the cast from PSUM-copy backlog;
row 112 = 1.0 for the fused "+1"), PE-transpose [112,128] chunks -> PSUM
[128,112] fp16, copy to SBUF (3 DVE + 1 ACT per tile), gram-matmul
accumulate into E_psum[112,112] (diag 7x7 blocks = per-q partial gram).
Gram matmuls trail the transposes by a few chunks so the in-order PE queue
never stalls.  fp16 is safe: top-2 energy gaps are >100 while fp16 gram
error is ~+-2.

Energy -> attention with one DVE op + two tiny PE matmuls (every engine
access starts at partition 0, per the BIR partition-alignment rule):
e7 = S^T (M o E) S where M is a block-diag ones mask and S[7q+n, n] = 1.
S, M, and the W template (zeros + ones row) are HOST-CONSTANT extra
inputs -- no on-chip staging.  Softmax on-chip, then the block-diag
W[113,112] (16 copies of (gamma*att)^T + ones row) is built with a 2-DMA
DRAM bounce on the SP queue.  x1 tiles prefetch during the pass-A tail so
HBM never idles through the softmax gap.

Pass B: out_psum[112,512] = W.T @ Xh slices (fp16, weights resident), then
y = out_psum * x1 multiplied IN-PLACE into the x1 tile on DVE, DMA'd out
on the gpsimd queue while x1 streams on SP/ACT.  x2 is read from HBM
exactly once; total HBM traffic = 3 * 29.4 MB per core.
"""

import sys

import numpy as np

try:
    import concourse.bass as bass
except ImportError:  # grading env fallback
    sys.path.insert(0, "/opt/trn_rl_repo")
    import concourse.bass as bass

from contextlib import ExitStack

import concourse.bacc as bacc
import concourse.tile as tile
from concourse import mybir
from concourse.bass_utils import run_bass_kernel_spmd
from concourse.masks import make_identity

F32 = mybir.dt.float32
F16 = mybir.dt.float16

B = 8
NN = 7              # attention dim
Q = 16              # d-runs per channel
P = NN * Q          # 112 partitions of (q, m)
PK = P + 1          # +1 ones row for the fused "+1"
FS = 2048           # stream tile free dim (8KB DRAM lines)
FM = 512            # matmul slice free dim (one PSUM bank)
D_FULL = 64 * 128 * 128
N_CORES = 8
PIPE = 8            # gram matmul trails transposes by this many chunks
NXT = 4             # pass-A stream slots
NPF = 5             # x1 tiles in flight (prefetch depth)


def const_inputs() -> dict:
    """Host-constant extra inputs: selector S, block-diag mask M, W template."""
    s = np.zeros((P, NN), dtype=np.float32)
    for q in range(Q):
        for n in range(NN):
            s[NN * q + n, n] = 1.0
    m = np.zeros((P, P), dtype=np.float32)
    for q in range(Q):
        m[NN * q:NN * (q + 1), NN * q:NN * (q + 1)] = 1.0
    w = np.zeros((PK, P), dtype=np.float16)
    w[P, :] = 1.0
    return {"cs": s, "cm": m, "cw": w}


def build_nc(d_total=D_FULL):
    assert d_total % (Q * FS) == 0
    ws = d_total // (Q * FS)          # stream tiles (32 at full size)
    cpt = FS // 128                   # transpose chunks per stream tile (16)
    mpt = FS // FM                    # matmul slices per stream tile (4)
    n_gram = ws * cpt

    nc = bacc.Bacc("TRN2", target_bir_lowering=False, debug=False)
    x1 = nc.dram_tensor("x1", [NN, d_total], F32, kind="ExternalInput")
    x2 = nc.dram_tensor("x2", [NN, d_total], F32, kind="ExternalInput")
    gm = nc.dram_tensor("gamma", [1], F32, kind="ExternalInput")
    cs = nc.dram_tensor("cs", [P, NN], F32, kind="ExternalInput")
    cm = nc.dram_tensor("cm", [P, P], F32, kind="ExternalInput")
    cw = nc.dram_tensor("cw", [PK, P], F16, kind="ExternalInput")
    y = nc.dram_tensor("y", [NN, d_total], F32, kind="ExternalOutput")

    x2v = x2[:].rearrange("m (q w f) -> q m w f", q=Q, w=ws, f=FS)
    x1v = x1[:].rearrange("m (q w f) -> q m w f", q=Q, w=ws, f=FS)
    yv = y[:].rearrange("m (q w f) -> q m w f", q=Q, w=ws, f=FS)

    with tile.TileContext(nc) as tc, ExitStack() as ctx:
        consts = ctx.enter_context(tc.tile_pool(name="consts", bufs=1))
        cache = ctx.enter_context(tc.tile_pool(name="cache", bufs=1))
        xs = ctx.enter_context(tc.tile_pool(name="xs", bufs=1))
        x1s = ctx.enter_context(tc.tile_pool(name="x1s", bufs=NPF))
        tsb = ctx.enter_context(tc.tile_pool(name="tsb", bufs=5))
        small = ctx.enter_context(tc.tile_pool(name="small", bufs=1))
        tps = ctx.enter_context(tc.tile_pool(name="tps", bufs=5, space="PSUM"))
        eps = ctx.enter_context(tc.tile_pool(name="eps", bufs=1, space="PSUM"))
        ops = ctx.enter_context(tc.tile_pool(name="ops", bufs=2, space="PSUM"))

        # ---------------- t=0 constants (all off the critical path) ---------
        ident = consts.tile([P, P], F16)
        make_identity(nc, ident)       # gpsimd; done before the first cast
        ones = consts.tile([1, FM], F16)
        nc.vector.memset(ones[:], 1.0)
        ones_bc = bass.AP(
            tensor=ones.tensor, offset=ones.offset,
            ap=[list(ones.ap[0]), [0, FS // FM], [1, FM]],
        )
        S = consts.tile([P, NN], F32)
        M = consts.tile([P, P], F32)
        wt = consts.tile([PK, P], F16)
        # consts pulled at t=0 on gpsimd: its triggers hide inside the
        # fixed ~8us framework preamble before the first x2 line lands
        nc.gpsimd.dma_start(out=S[:], in_=cs[:])
        nc.gpsimd.dma_start(out=M[:], in_=cm[:])
        nc.gpsimd.dma_start(out=wt[:], in_=cw[:])   # zeros + ones row
        a32 = small.tile([32, 32], F32)
        nc.vector.memset(a32[:], 0.0)
        gsb = small.tile([NN, 1], F32)
        nc.gpsimd.dma_start(
            out=gsb[:],
            in_=bass.AP(tensor=gm[:].tensor, offset=0, ap=[[0, NN], [1, 1]]),
        )

        E = eps.tile([P, P], F32)
        xh = [cache.tile([PK, FS], F16, name=f"xh{w}", tag=f"xh{w}")
              for w in range(ws)]

        # ~5us of dummy matmuls so the PE HAM clock-gate opens before the
        # real pass-A stream arrives (and stays open)
        for _ in range(48):
            wm = ops.tile([P, P], F32, tag="op")
            nc.tensor.matmul(wm[:], lhsT=ident[:], rhs=ident[:],
                             start=True, stop=True)

        # ---------------- pass A: stream x2, cast, transpose, gram ----------
        pend = []          # tt APs awaiting gram matmul
        gi = 0             # gram matmuls emitted

        def emit_gram(tt_ap):
            nonlocal gi
            nc.tensor.matmul(E[:], lhsT=tt_ap, rhs=tt_ap,
                             start=(gi == 0), stop=(gi == n_gram - 1))
            gi += 1

        GRP = 4                       # transpose chunks batched per PSUM bank
        # persistent stream slots: row 112 holds 1.0, copied along by the cast
        xts = [xs.tile([PK, FS], F32, name=f"xt{i}", tag=f"xt{i}")
               for i in range(NXT)]
        for i in range(NXT):
            nc.gpsimd.dma_start(out=xts[i][P:PK, :], in_=ones_bc)

        x1ts = [None] * ws            # x1 tiles, allocated at prefetch time

        def emit_x1_load(w):
            x1ts[w] = x1s.tile([P, FS], F32, name=f"x1t{w}", tag="x1t")
            x1e = nc.scalar if w % 2 == 0 else nc.sync
            x1e.dma_start(out=x1ts[w][:], in_=x1v[:, :, w, :])

        # copies of tile w-1 are emitted AFTER cast(w) so the DVE stream
        # never holds the next cast hostage to PE progress (the engines
        # run ahead; tsb/tps ring depth absorbs the skew)
        tpq = []           # transpose PSUM groups awaiting copy

        def emit_copies():
            while tpq:
                g, tp = tpq.pop(0)
                tt = tsb.tile([128, GRP * P], F16)
                if g < 2:
                    nc.vector.tensor_copy(out=tt[:], in_=tp[:])
                else:
                    nc.scalar.copy(tt[:], tp[:])
                for k in range(GRP):
                    pend.append(tt[:, k * P:(k + 1) * P])
                while len(pend) > PIPE:
                    emit_gram(pend.pop(0))

        for w in range(ws):
            xt = xts[w % NXT]
            dmae = nc.sync if w % 2 == 0 else nc.scalar
            dmae.dma_start(out=xt[0:P, :], in_=x2v[:, :, w, :])
            nc.vector.tensor_copy(out=xh[w][:], in_=xt[:])     # f32 -> f16
            if w >= ws - NPF:
                emit_x1_load(w - (ws - NPF))   # prefetch x1 behind x2 tail
            emit_copies()                      # tile w-1's PSUM groups
            for g in range(cpt // GRP):
                tp = tps.tile([128, GRP * P], F16)
                for k in range(GRP):
                    c = g * GRP + k
                    nc.tensor.transpose(
                        tp[:, k * P:(k + 1) * P],
                        xh[w][0:P, c * 128:(c + 1) * 128], ident[:])
                tpq.append((g, tp))
        emit_copies()
        for tt in pend:
            emit_gram(tt)
        pend = []

        # ---------------- energy -> attention -> weights --------------------
        # e7 = S^T (M o E) S sums exactly the diag 7x7 blocks of E; the
        # mask + two tiny selector matmuls keep every engine access
        # starting at partition 0 (BIR partition-alignment rule).
        ME = small.tile([P, P], F32)
        nc.vector.tensor_mul(ME[:], E[:], M[:])
        U = ops.tile([P, NN], F32, tag="op")
        nc.tensor.matmul(U[:], lhsT=ME[:], rhs=S[:], start=True, stop=True)
        U_sb = small.tile([P, NN], F32)
        nc.scalar.copy(U_sb[:], U[:])
        e7 = ops.tile([NN, NN], F32, name="e7", tag="op")
        nc.tensor.matmul(e7[:], lhsT=S[:], rhs=U_sb[:], start=True, stop=True)

        mn = small.tile([NN, 1], F32)
        nc.vector.tensor_reduce(
            out=mn[:], in_=e7[:], axis=mybir.AxisListType.X,
            op=mybir.AluOpType.min,
        )
        ex = small.tile([NN, NN], F32)
        nc.scalar.activation(
            out=ex[:], in_=e7[:], func=mybir.ActivationFunctionType.Exp,
            bias=mn[:], scale=-1.0,
        )                                              # exp(rowmin - E)
        z = small.tile([NN, 1], F32)
        nc.vector.tensor_reduce(
            out=z[:], in_=ex[:], axis=mybir.AxisListType.X,
            op=mybir.AluOpType.add,
        )
        r = small.tile([NN, 1], F32)
        nc.vector.reciprocal(r[:], z[:])
        rg = small.tile([NN, 1], F32)
        nc.vector.tensor_mul(rg[:], r[:], gsb[:])      # gamma / Z_n
        nc.vector.tensor_scalar_mul(a32[0:NN, 0:NN], ex[:], rg[:])  # gamma*att
        at32 = small.tile([32, 32], F32)
        nc.vector.transpose(at32[:], a32[:])           # (gamma*att)^T
        at16 = small.tile([32, 32], F16)
        nc.vector.tensor_copy(out=at16[:], in_=at32[:])
        # block-diag W: 16 tiny SBUF->SBUF DMAs straight into wt (template
        # with zeros + ones row preloaded at t=0; no DRAM roundtrip)
        for q in range(Q):
            sl = slice(NN * q, NN * (q + 1))
            weng = nc.sync if q % 2 == 0 else nc.scalar
            weng.dma_start(out=wt[sl, sl], in_=at16[0:NN, 0:NN])

        for _ in range(4):             # keep PE p-state up through the gap
            wm2 = ops.tile([P, P], F32, tag="op")
            nc.tensor.matmul(wm2[:], lhsT=ident[:], rhs=ident[:],
                             start=True, stop=True)

        # ---------------- pass B: out = W.T @ Xh; y = out * x1 (in place) ---
        for w in range(ws):
            if w + NPF < ws:
                emit_x1_load(w + NPF)
            x1t = x1ts[w]
            for j in range(mpt):
                sl = slice(j * FM, (j + 1) * FM)
                op = ops.tile([P, FM], F32, tag="op")
                nc.tensor.matmul(op[:], lhsT=wt[:], rhs=xh[w][:, sl],
                                 start=True, stop=True)
                nc.vector.tensor_mul(x1t[:, sl], op[:], x1t[:, sl])
            nc.gpsimd.dma_start(out=yv[:, :, w, :], in_=x1t[:])

    nc.compile()
    return nc


_NC_CACHE = {}


def _get_nc(d_total=D_FULL):
    if d_total not in _NC_CACHE:
        _NC_CACHE[d_total] = build_nc(d_total)
    return _NC_CACHE[d_total]


def kernel(x1: np.ndarray, x2: np.ndarray, gamma: np.ndarray) -> np.ndarray:
    b, n, c, h, w = x1.shape
    assert (b, n) == (B, NN)
    d = c * h * w
    x1r = np.ascontiguousarray(x1.reshape(b, n, d)).astype(np.float32, copy=False)
    x2r = np.ascontiguousarray(x2.reshape(b, n, d)).astype(np.float32, copy=False)
    g = np.asarray(gamma, dtype=np.float32).reshape(1)

    nc = _get_nc(d)
    cst = const_inputs()
    in_maps = [
        {"x1": x1r[i], "x2": x2r[i], "gamma": g, **cst} for i in range(N_CORES)
    ]
    res = run_bass_kernel_spmd(nc, in_maps, list(range(N_CORES)))
    out = np.stack([res.results[i]["y"] for i in range(N_CORES)], axis=0)
    return out.reshape(b, n, c, h, w).astype(np.float32, copy=False)
